# revision 72
# baseline (speedup 1.0000x reference)
"""Trainium2 Bass kernel for a 2-layer GNN (scatter-mean message passing)
with BN+ReLU and a 2-layer MLP classifier, distributed over 8 NeuronCores.

Math (reference):
    x2 = relu(bn1(mean_agg(x @ W1 + b1)))
    x3 = relu(bn2(mean_agg(x2 @ W2 + b2)))
    out = relu(x3 @ Wc1 + bc1) @ Wc2 + bc2

Device strategy (per core, nodes globally degree-sorted and round-robined
across cores, 12500 dst nodes per core):
  * BN affine folds into W/b (host).  Linear maps commute with the mean
    aggregation, so each layer is: gather source rows -> S-matmul
    aggregation -> apply W' -> 1/cnt scale -> bias+relu.
  * Aggregation: for each 256-dst window, dma_gather pulls the window's
    source rows (4 source-quarter runs for int16 indices), then TensorE
    computes aggT[:, c0:c0+nc] += G_tile^T @ S_tile with host-built
    per-tile-compacted fp8 S tiles streamed from HBM.
  * Gather tuning: 4 SWDGE queues round-robin (parallel Q7 descriptor
    generation); calls sized GMAX=512 so 2+ fit each 64-desc ring
    (gen/drain pipelining); per-call real counts streamed via
    reg_load + num_idxs_reg with -1 idx padding so the Q7 skips
    descriptors for run-tail padding (cross-core imbalance).
  * The fp16 node table for layer 1 is host-precast (full copy per
    core); layer-2's x2 table is AllGather'ed in 14 chunks in the
    [chunk][core][rows] layout so shipping overlaps the layer-1 tail,
    and layer-2 gathers only wait on the chunks their quarter spans.
  * Output is produced transposed [2, dst]; host reassembles [N, 2].
"""

import os
import numpy as np

N = 100000
DIN = 128
HID = 128
EPS = 1e-5
NCORES = 8
SHARD = 12500
WCOLS = 256
NWIN = 49                     # ceil(12500/256)
SHARD_PAD = NWIN * WCOLS      # 12544
TROWS = SHARD_PAD * NCORES    # 100352 table rows
NQ = 4
QROWS = TROWS // NQ           # 25088 (< 32768: int16-safe)
KAG = int(os.environ.get("GNN_KAG", 7))   # AllGather chunks
CH_W = SHARD_PAD // KAG       # rows per core per chunk (must be k*128)
CH_ROWS = CH_W * NCORES       # table rows per chunk


def _table_row(c, p):
    """Table row of (core c, sorted position p) in chunked layout."""
    k = p // CH_W
    return k * CH_ROWS + c * CH_W + p % CH_W

_CACHE = {}
LAST_EXEC_NS = None


def _ceil128(a):
    return ((a + 127) // 128) * 128


def _fold_bn(Wm, b, g, be, m, v):
    a = (g / np.sqrt(v + EPS)).astype(np.float64)
    Wp = (Wm.astype(np.float64) * a[None, :]).astype(np.float32)
    bp = (b.astype(np.float64) * a + be - m.astype(np.float64) * a).astype(
        np.float32)
    return Wp, bp


def _prep(edge_index):
    """Host-side graph preprocessing. Returns (meta, per_core_arrays)."""
    src = np.concatenate([edge_index[0], np.arange(N, dtype=edge_index.dtype)])
    dst = np.concatenate([edge_index[1], np.arange(N, dtype=edge_index.dtype)])
    src = src.astype(np.int64)
    dst = dst.astype(np.int64)
    cnt = np.bincount(dst, minlength=N)
    winv = (1.0 / cnt).astype(np.float32)

    # global degree sort, round-robin across cores: every core's window w
    # covers the same degree band, so per-(window, quarter) run lengths are
    # near-equal across cores (less ceil128 / max-across-core padding) and
    # windows stay degree-homogeneous (small S column spans).
    order_g = np.argsort(-cnt, kind="stable")    # global rank -> node
    core_of = np.empty(N, np.int64)
    spos_g = np.empty(N, np.int64)               # node -> position in core
    ranks = np.arange(N)
    core_of[order_g] = ranks % NCORES
    spos_g[order_g] = ranks // NCORES
    orders = [order_g[c::NCORES] for c in range(NCORES)]  # pos -> global node

    core = core_of[dst]
    local = spos_g[dst]
    win = local // WCOLS
    col = local % WCOLS
    rowid = _table_row(core_of[src], spos_g[src])
    q = rowid // QROWS
    idxl = (rowid % QROWS).astype(np.int16)

    # per (core, window, quarter) counts -> global run sizes R [NWIN, NQ]
    key = (core * NWIN + win) * NQ + q
    counts = np.bincount(key, minlength=NCORES * NWIN * NQ).reshape(
        NCORES, NWIN, NQ)
    R = counts.max(axis=0)
    R = np.where(R > 0, _ceil128(R), 0)          # [NWIN, NQ]
    runbase = np.zeros((NWIN, NQ), np.int64)     # slot offset within window
    runbase[:, 1:] = np.cumsum(R[:, :-1], axis=1)
    winslots = R.sum(axis=1)                     # [NWIN]
    winbase = np.zeros(NWIN, np.int64)
    winbase[1:] = np.cumsum(winslots[:-1])
    tslots = int(winslots.sum())
    ntiles_w = (winslots // 128).astype(np.int64)
    tilebase = np.zeros(NWIN, np.int64)          # global tile index base
    tilebase[1:] = np.cumsum(ntiles_w[:-1])
    ntiles = int(ntiles_w.sum())

    # per-core slot assignment
    per_core = []
    gslot_all = np.empty(len(dst), np.int64)
    for c in range(NCORES):
        m = np.flatnonzero(core == c)
        order = np.lexsort((col[m], q[m], win[m]))
        me = m[order]
        runkey = win[me] * NQ + q[me]
        # rank within each run (runkey is sorted)
        change = np.r_[True, runkey[1:] != runkey[:-1]]
        starts = np.flatnonzero(change)
        lens = np.diff(np.r_[starts, len(runkey)])
        rank = np.arange(len(runkey)) - np.repeat(starts, lens)
        gslot = winbase[win[me]] + runbase[win[me], q[me]] + rank
        gslot_all[me] = gslot
        per_core.append(me)

    # tile column spans (across all cores)
    gtile_all = gslot_all // 128
    col_min = np.full(ntiles, WCOLS, np.int64)
    col_max = np.full(ntiles, -1, np.int64)
    np.minimum.at(col_min, gtile_all, col)
    np.maximum.at(col_max, gtile_all, col)
    col_min = np.minimum(col_min, WCOLS - 1)
    col_max = np.maximum(col_max, col_min)

    # per-tile S column spans (8-aligned) — tighter than per-window max
    span = col_max - col_min + 1
    nc_per_tile = np.minimum(WCOLS, ((span + 7) // 8) * 8)
    c0 = np.minimum(col_min, WCOLS - nc_per_tile)
    c0 = np.maximum(c0, 0)

    scolbase = np.zeros(ntiles, np.int64)   # S column base per global tile
    scolbase[1:] = np.cumsum(nc_per_tile[:-1])
    stot = int(nc_per_tile.sum())

    # per-core packed arrays
    import ml_dtypes
    # per-(core, window, quarter) real counts for dynamic num_idxs_reg
    rcnt = counts  # [NCORES, NWIN, NQ]
    idx_arrs, s_arrs, cinv_arrs = [], [], []
    for c in range(NCORES):
        me = per_core[c]
        gslot = gslot_all[me]
        # -1 padding: with dynamic num_idxs_reg (GNN_DYN=1) the Q7 gather
        # trims trailing negative idxs per call, skipping descriptor
        # generation + DMA for run-tail padding. The register value MUST
        # equal the per-call count of non-negative idxs.
        import os as _os
        pad = -1 if _os.environ.get("GNN_DYN", "1") == "1" else 0
        idx_flat = np.full(tslots, pad, np.int16)
        idx_flat[gslot] = idxl[me]
        idx16 = np.ascontiguousarray(
            idx_flat.reshape(tslots // 16, 16).T)          # [16, tslots/16]
        idx_packed = np.tile(idx16, (8, 1))                 # [128, ...]

        if _os.environ.get("GNN_IND", "0") == "1":
            # int32 global-rowid stream for indirect_dma_start, permuted
            # per (window, quarter) run so the out AP's C-order flat
            # traversal [128, Tq, 128] (partition-major) matches:
            # stream[j] = rowid of slot rb + (j % Tq)*128 + (j // Tq).
            rid = np.zeros(tslots, np.int32)
            rid[gslot] = (q[me] * QROWS + idxl[me].astype(np.int64)).astype(
                np.int32)
            stream = np.empty(tslots, np.int32)
            for w in range(NWIN):
                for qq2 in range(NQ):
                    rq = int(R[w, qq2])
                    if rq == 0:
                        continue
                    b = int(winbase[w]) + int(runbase[w, qq2])
                    seg = rid[b:b + rq].reshape(rq // 128, 128)
                    stream[b:b + rq] = seg.T.ravel()
            idx_packed = stream.reshape(1, -1)

        s_arr = np.zeros((128, stot), ml_dtypes.float8_e4m3)
        gt = gslot // 128
        p = gslot % 128
        sc = scolbase[gt] + (col[me] - c0[gt])
        s_arr[p, sc] = 1.0
        idx_arrs.append(idx_packed)
        s_arrs.append(s_arr)
        civ = np.zeros(SHARD_PAD, np.float32)
        civ[:SHARD] = winv[orders[c]]
        cinv_arrs.append(civ)

    meta = dict(
        R=R, runbase=runbase, winslots=winslots, winbase=winbase,
        tslots=tslots, ntiles_w=ntiles_w, tilebase=tilebase,
        nc_t=nc_per_tile, c0=c0, scolbase=scolbase, stot=stot,
        orders=orders, cinv=cinv_arrs, rcnt=rcnt,
    )
    return meta, idx_arrs, s_arrs


def _build(meta):
    import concourse.bass as bass
    import concourse.bacc as bacc
    import concourse.tile as tile
    from concourse import mybir
    from concourse.masks import make_identity
    from concourse.tile_rust import add_dep_helper

    f32 = mybir.dt.float32
    f16 = mybir.dt.float16
    i16 = mybir.dt.int16
    AF = mybir.ActivationFunctionType

    R = meta["R"]; runbase = meta["runbase"]; winslots = meta["winslots"]
    winbase = meta["winbase"]; tslots = meta["tslots"]
    ntiles_w = meta["ntiles_w"]; tilebase = meta["tilebase"]
    nc_t = meta["nc_t"]; c0 = meta["c0"]; scolbase = meta["scolbase"]
    stot = meta["stot"]

    nqueues = int(os.environ.get("GNN_NQUEUES", 4))
    scratch = int(os.environ.get("GNN_SCRATCH", 16384))
    nc = bacc.Bacc("TRN2", target_bir_lowering=False, debug=False,
                   num_devices=NCORES, num_swdge_queues=nqueues,
                   dynamic_dma_scratch_size=scratch)

    ind = os.environ.get("GNN_IND", "0") == "1"
    dyn = os.environ.get("GNN_DYN", "1") == "1" and not ind
    GMAX = int(os.environ.get("GNN_GMAX", 512))
    ncalls = 0
    for w in range(NWIN):
        if winslots[w] == 0:
            continue
        for qq in range(4):
            rq = int(R[w, qq])
            ncalls += (rq + GMAX - 1) // GMAX

    xf_in = nc.dram_tensor("xf", [TROWS, DIN], f16, kind="ExternalInput")
    if ind:
        idx_in = nc.dram_tensor("idx", [1, tslots], mybir.dt.int32,
                                kind="ExternalInput")
    else:
        idx_in = nc.dram_tensor("idx", [128, tslots // 16], i16,
                                kind="ExternalInput")
    gcnt_in = None
    ncalls8 = ((max(ncalls, 1) + 7) // 8) * 8
    if dyn:
        gcnt_in = nc.dram_tensor("gcnt", [1, ncalls8], mybir.dt.int32,
                                 kind="ExternalInput")
    f8 = mybir.dt.float8e4
    s_in = nc.dram_tensor("sv", [128, stot], f8, kind="ExternalInput")
    cinv_in = nc.dram_tensor("cinv", [1, SHARD_PAD], f32,
                             kind="ExternalInput")
    w1_in = nc.dram_tensor("w1", [DIN, HID], f16, kind="ExternalInput")
    w2_in = nc.dram_tensor("w2", [HID, HID], f16, kind="ExternalInput")
    wc1_in = nc.dram_tensor("wc1", [HID, HID // 2], f16, kind="ExternalInput")
    wc2_in = nc.dram_tensor("wc2", [HID // 2, 2], f16, kind="ExternalInput")
    b1_in = nc.dram_tensor("b1", [HID, 1], f32, kind="ExternalInput")
    b2_in = nc.dram_tensor("b2", [HID, 1], f32, kind="ExternalInput")
    bc1_in = nc.dram_tensor("bc1", [HID // 2, 1], f32, kind="ExternalInput")
    bc2_in = nc.dram_tensor("bc2", [2, 1], f32, kind="ExternalInput")
    out_dram = nc.dram_tensor("outT", [2, SHARD_PAD], f32,
                              kind="ExternalOutput")

    x2_shard = nc.dram_tensor("x2_shard", [SHARD_PAD, DIN], f16)
    x2_full = nc.dram_tensor("x2_full", [TROWS, DIN], f16,
                             addr_space="Shared")

    groups = [list(range(NCORES))]

    with tile.TileContext(nc) as tc:
        with (
            tc.tile_pool(name="cst", bufs=1) as cst,
            tc.tile_pool(name="stage", bufs=5) as stage,
            tc.tile_pool(name="meta_p", bufs=5) as meta_p,
            tc.tile_pool(name="small", bufs=3) as small,
            tc.tile_pool(name="pacc", bufs=2, space="PSUM") as pacc,
            tc.tile_pool(name="pmm", bufs=2, space="PSUM") as pmm,
            tc.tile_pool(name="ptp", bufs=2, space="PSUM") as ptp,
        ):
            # constants
            ident = cst.tile([128, 128], f16)
            make_identity(nc, ident)
            w1_t = cst.tile([DIN, HID], f16)
            nc.sync.dma_start(out=w1_t, in_=w1_in[:])
            w2_t = cst.tile([HID, HID], f16)
            nc.sync.dma_start(out=w2_t, in_=w2_in[:])
            wc1_t = cst.tile([HID, HID // 2], f16)
            nc.sync.dma_start(out=wc1_t, in_=wc1_in[:])
            wc2_t = cst.tile([HID // 2, 2], f16)
            nc.sync.dma_start(out=wc2_t, in_=wc2_in[:])
            b1_t = cst.tile([HID, 1], f32)
            nc.sync.dma_start(out=b1_t, in_=b1_in[:])
            b2_t = cst.tile([HID, 1], f32)
            nc.sync.dma_start(out=b2_t, in_=b2_in[:])
            bc1_t = cst.tile([HID // 2, 1], f32)
            nc.sync.dma_start(out=bc1_t, in_=bc1_in[:])
            bc2_t = cst.tile([2, 1], f32)
            nc.sync.dma_start(out=bc2_t, in_=bc2_in[:])

            # prime the gather staging buffers: with -1 idx trimming, padded
            # slots are never written by the DMA, so zero both rotating
            # buffers once to guarantee finite values under the S zeros.
            maxws = int(winslots.max())
            for _ in range(5):
                gprime = stage.tile([128, maxws // 128, DIN], f16, tag="g")
                nc.vector.memset(gprime[:], 0.0)

            # the fp16 node table arrives pre-cast from the host (full copy
            # on every core) — no phase-0 cast or AllGather needed.
            nocc = bool(os.environ.get("GNN_NOCC"))
            nwin_lim = int(os.environ.get("GNN_NWIN", NWIN))
            cut = int(os.environ.get("GNN_CUT", 99))

            qctr = [0]
            gcnt_t = None
            regs = []
            if dyn:
                gcnt_t = cst.tile([1, ncalls8], mybir.dt.int32)
                nc.sync.dma_start(out=gcnt_t, in_=gcnt_in[:])
                # 8 registers, batch-loaded 8 counts per reg_load: halves
                # the Pool-sequencer instruction count vs one load/gather.
                regs = [nc.gpsimd.register(f"gcnt{i}").__enter__()
                        for i in range(8)]
            callctr = [0]

            def layer(table, cc_by_q, wt, bt, is_last, on_chunk=None):
                """One GNN layer over all windows. cc_by_q[qq] lists the
                collective deps that must complete before quarter qq's
                gathers may read the table. on_chunk(k, writes) is called
                as soon as AllGather chunk k's rows are fully written, so
                the collective is emitted mid-loop and overlaps the
                remaining windows (the Pool queue executes in order)."""
                writes = {}
                chunks_done = [0]
                for w in range(min(NWIN, nwin_lim)):
                    ws = int(winslots[w])
                    if ws == 0:
                        continue
                    tw = int(ntiles_w[w])
                    t0 = int(tilebase[w])
                    sb0 = int(scolbase[t0])
                    swid = int(nc_t[t0:t0 + tw].sum())
                    # idx + S streams for this window
                    if ind:
                        idx_t = meta_p.tile([1, ws], mybir.dt.int32,
                                            tag="idx")
                        nc.sync.dma_start(
                            out=idx_t,
                            in_=idx_in[0:1, int(winbase[w]):
                                       int(winbase[w]) + ws])
                    else:
                        idx_t = meta_p.tile([128, ws // 16], i16, tag="idx")
                        nc.sync.dma_start(
                            out=idx_t,
                            in_=idx_in[:, int(winbase[w]) // 16:
                                       (int(winbase[w]) + ws) // 16])
                    s_t = meta_p.tile([128, swid], f8, tag="s")
                    nc.sync.dma_start(
                        out=s_t, in_=s_in[:, sb0:sb0 + swid])

                    g = stage.tile([128, ws // 128, DIN], f16, tag="g")
                    if ind:
                        # one streaming indirect gather per quarter run;
                        # idxs are int32 global rowids permuted so the out
                        # AP's flat order receives them correctly.
                        for qq in range(NQ):
                            rq = int(R[w, qq])
                            if rq == 0:
                                continue
                            rb = int(runbase[w, qq])
                            gi = nc.gpsimd.indirect_dma_start(
                                out=g[:, rb // 128:(rb + rq) // 128, :],
                                out_offset=None,
                                in_=table[:],
                                in_offset=bass.IndirectOffsetOnAxis(
                                    ap=idx_t[0:1, rb:rb + rq], axis=0),
                            )
                            for cd in cc_by_q[qq]:
                                add_dep_helper(gi.ins, cd.ins, True,
                                               "gather after table ready")
                        if cut < 2:
                            continue
                    for qq in range(NQ if not ind else 0):
                        rq = int(R[w, qq])
                        rb0 = int(runbase[w, qq])
                        for ck in range(0, rq, GMAX):
                            rc = min(GMAX, rq - ck)
                            rb = rb0 + ck
                            qn = qctr[0] % nqueues
                            if dyn:
                                ci = callctr[0] % ncalls
                                if ci % 8 == 0:
                                    nc.gpsimd.reg_load(
                                        regs, gcnt_t[0:1, ci:ci + 8])
                                nreg = regs[ci % 8]
                            else:
                                nreg = rc
                            gi = nc.gpsimd.dma_gather(
                                out_ap=g[:, rb // 128:
                                         (rb + rc + 127) // 128, :],
                                in_ap=table[qq * QROWS:(qq + 1) * QROWS, :],
                                idxs_ap=idx_t[:, rb // 16:
                                              (rb + rc + 15) // 16],
                                num_idxs=rc,
                                num_idxs_reg=nreg,
                                elem_size=DIN,
                                queue_num=qn,
                                single_packet=os.environ.get(
                                    "GNN_SP", "1") == "1",
                            )
                            qctr[0] += 1
                            callctr[0] += 1
                            for cd in cc_by_q[qq]:
                                add_dep_helper(gi.ins, cd.ins, True,
                                               "gather after table ready")

                    if cut < 2:
                        continue
                    acc = pacc.tile([128, WCOLS], f32, space="PSUM",
                                    tag="acc")
                    nc.vector.memset(acc[:], 0.0)
                    for t in range(tw):
                        cc_0 = int(c0[t0 + t])
                        nct = int(nc_t[t0 + t])
                        soff = int(scolbase[t0 + t]) - sb0
                        nc.tensor.matmul(
                            acc[:, cc_0:cc_0 + nct],
                            lhsT=g[:, t, :],
                            rhs=s_t[:, soff:soff + nct],
                            start=False, stop=(t == tw - 1),
                            skip_group_check=True,
                        )
                    if cut < 3:
                        continue
                    aggT = small.tile([128, WCOLS], f16, tag="aggT")
                    nc.vector.tensor_copy(out=aggT[:], in_=acc[:])

                    msgT = pmm.tile([128, WCOLS], f32, space="PSUM",
                                    tag="mm")
                    nc.tensor.matmul(msgT[:], lhsT=wt, rhs=aggT[:],
                                     start=True, stop=True)
                    cb = small.tile([128, WCOLS], f32, tag="cb")
                    nc.sync.dma_start(
                        out=cb,
                        in_=bass.AP(tensor=cinv_in[:].tensor,
                                    offset=w * WCOLS,
                                    ap=[[0, 128], [1, WCOLS]]))
                    mmf = small.tile([128, WCOLS], f32, tag="mmf")
                    nc.vector.tensor_mul(mmf[:], msgT[:], cb[:])
                    xoT = small.tile([128, WCOLS], f16, tag="xoT")
                    nc.scalar.activation(out=xoT[:], in_=mmf[:],
                                         func=AF.Relu, bias=bt, scale=1.0)

                    if cut < 4:
                        continue
                    if not is_last:
                        # transpose to row-major and store shard rows
                        for h in range(2):
                            tp = ptp.tile([128, 128], f16, space="PSUM",
                                          tag="tp")
                            nc.tensor.transpose(
                                out=tp[:], in_=xoT[:, h * 128:(h + 1) * 128],
                                identity=ident[:])
                            xrows = small.tile([128, 128], f16, tag="xrows")
                            nc.vector.tensor_copy(out=xrows[:], in_=tp[:])
                            wi = nc.sync.dma_start(
                                out=x2_shard[w * WCOLS + h * 128:
                                             w * WCOLS + (h + 1) * 128, :],
                                in_=xrows[:])
                            writes.setdefault(
                                (w * WCOLS + h * 128) // CH_W,
                                []).append(wi)
                    else:
                        # fused classifier on xoT = x3^T [feat, dst]
                        y1 = pmm.tile([HID // 2, WCOLS], f32, space="PSUM",
                                      tag="mm")
                        nc.tensor.matmul(y1[:], lhsT=wc1_t, rhs=xoT[:],
                                         start=True, stop=True)
                        y1T = small.tile([HID // 2, WCOLS], f16, tag="y1T")
                        nc.scalar.activation(out=y1T[:], in_=y1[:],
                                             func=AF.Relu, bias=bc1_t,
                                             scale=1.0)
                        lo = pmm.tile([2, WCOLS], f32, space="PSUM",
                                      tag="mm")
                        nc.tensor.matmul(lo[:], lhsT=wc2_t, rhs=y1T[:],
                                         start=True, stop=True)
                        loT = small.tile([2, WCOLS], f32, tag="loT")
                        nc.scalar.activation(out=loT[:], in_=lo[:],
                                             func=AF.Identity, bias=bc2_t,
                                             scale=1.0)
                        writes.setdefault(-1, []).append(nc.sync.dma_start(
                            out=out_dram[:, w * WCOLS:(w + 1) * WCOLS],
                            in_=loT[:]))
                    if on_chunk is not None:
                        # emit an AllGather chunk only a couple of windows
                        # after its rows were written: its deps are then
                        # already satisfied, so it dispatches immediately
                        # instead of parking in the Pool wait queue (depth
                        # 4) and throttling gather dispatch.
                        kdone = min(max(w - 1, 0) * WCOLS // CH_W, KAG)
                        while chunks_done[0] < kdone:
                            on_chunk(chunks_done[0], writes)
                            chunks_done[0] += 1
                return writes

            phase = int(os.environ.get("GNN_PHASE", 2))
            reps = int(os.environ.get("GNN_REPS", 1))
            dummy = small.tile([2, WCOLS], f32, tag="loT")
            # quarter q of the table spans AllGather chunks qdep[q]
            qdep = []
            for q in range(NQ):
                k0 = q * QROWS // CH_ROWS
                k1 = ((q + 1) * QROWS - 1) // CH_ROWS
                qdep.append(list(range(k0, k1 + 1)))

            for _rep in range(reps):
                # chunked AllGather, emitted inline as soon as each chunk's
                # rows are written so the collective overlaps later windows
                # (the Pool queue executes in program order).
                ags = [None] * KAG

                def emit_ag(k, writes):
                    ag = nc.gpsimd.collective_compute(
                        "AllGather", mybir.AluOpType.bypass,
                        replica_groups=groups,
                        ins=[x2_shard[k * CH_W:(k + 1) * CH_W, :]],
                        outs=[x2_full[k * CH_ROWS:(k + 1) * CH_ROWS, :]])
                    for wi in writes.get(k, []):
                        add_dep_helper(ag.ins, wi.ins, True,
                                       "ag chunk after x2 writes")
                    ags[k] = ag

                inline_ag = os.environ.get("GNN_AGI", "1") == "1"
                if phase >= 1:
                    l1_writes = layer(xf_in, [[]] * NQ, w1_t, b1_t,
                                      is_last=False,
                                      on_chunk=emit_ag if inline_ag
                                      else None)
                if phase < 2:
                    continue
                for k in range(KAG):
                    if ags[k] is None:
                        emit_ag(k, l1_writes)
                assert all(a is not None for a in ags)
                cc_by_q = [[ags[k] for k in qdep[q]] for q in range(NQ)]
                layer(x2_full, cc_by_q, w2_t, b2_t, is_last=True)
            if phase < 2:
                # ensure output written so harness path works
                nc.vector.memset(dummy[:], 0.0)
                nc.sync.dma_start(out=out_dram[:, 0:WCOLS], in_=dummy[:])

    import time as _t
    t0 = _t.time()
    nc.compile()
    print(f"[kernel] bacc compile: {_t.time() - t0:.1f}s", flush=True)
    return nc


def _bench(nc, in_maps, iters=6):
    """Time pure device execution with device-resident inputs."""
    import time
    import jax
    import numpy as np_
    from jax.sharding import Mesh, PartitionSpec, NamedSharding
    from jax.experimental.shard_map import shard_map
    from concourse import bass2jax as b2j
    from concourse import mybir

    b2j.install_neuronx_cc_hook()
    in_names, out_names, out_avals, zero_outs = [], [], [], []
    partition_name = (nc.partition_id_tensor.name
                      if nc.partition_id_tensor else None)
    for alloc in nc.m.functions[0].allocations:
        if not isinstance(alloc, mybir.MemoryLocationSet):
            continue
        name = alloc.memorylocations[0].name
        if alloc.kind == "ExternalInput":
            if name != partition_name:
                in_names.append(name)
        elif alloc.kind == "ExternalOutput":
            out_names.append(name)
            shape = tuple(alloc.tensor_shape)
            dtype = mybir.dt.np(alloc.dtype)
            out_avals.append(jax.core.ShapedArray(shape, dtype))
            zero_outs.append(np_.zeros(shape, dtype))
    n_params = len(in_names)
    n_outs = len(out_avals)
    all_names = in_names + out_names + (
        [partition_name] if partition_name else [])
    donate = tuple(range(n_params, n_params + n_outs))

    def _body(*args):
        operands = list(args)
        if partition_name is not None:
            operands.append(b2j.partition_id_tensor())
        return tuple(b2j._bass_exec_p.bind(
            *operands, out_avals=tuple(out_avals), in_names=tuple(all_names),
            out_names=tuple(out_names), lowering_input_output_aliases=(),
            sim_require_finite=True, sim_require_nnan=True, nc=nc))

    devices = jax.devices()[:NCORES]
    mesh = Mesh(np_.asarray(devices), ("core",))
    in_specs = (PartitionSpec("core"),) * (n_params + n_outs)
    out_specs = (PartitionSpec("core"),) * n_outs
    fn = jax.jit(shard_map(_body, mesh=mesh, in_specs=in_specs,
                           out_specs=out_specs, check_rep=False),
                 donate_argnums=donate, keep_unused=True)
    sh = NamedSharding(mesh, PartitionSpec("core"))
    concat_in = [
        jax.device_put(np_.concatenate(
            [np_.asarray(in_maps[c][nm]) for c in range(NCORES)], axis=0), sh)
        for nm in in_names]
    times = []
    for it in range(iters):
        zs = [jax.device_put(
            np_.zeros((NCORES * z.shape[0], *z.shape[1:]), z.dtype), sh)
            for z in zero_outs]
        for z in zs:
            z.block_until_ready()
        t0 = time.perf_counter()
        outs = fn(*concat_in, *zs)
        for o in outs:
            o.block_until_ready()
        times.append(time.perf_counter() - t0)
    best = min(times[1:]) if len(times) > 1 else times[0]
    print(f"[bench] exec times (s): {[f'{t:.4f}' for t in times]}")
    return int(best * 1e9)


def kernel(x, edge_index, W1, b1, g1, be1, m1, v1, W2, b2, g2, be2, m2, v2,
           Wc1, bc1, Wc2, bc2):
    global LAST_EXEC_NS
    from concourse.bass_utils import run_bass_kernel_spmd

    x = np.asarray(x, dtype=np.float32)
    edge_index = np.asarray(edge_index)

    W1p, b1p = _fold_bn(np.asarray(W1), np.asarray(b1), np.asarray(g1),
                        np.asarray(be1), np.asarray(m1), np.asarray(v1))
    W2p, b2p = _fold_bn(np.asarray(W2), np.asarray(b2), np.asarray(g2),
                        np.asarray(be2), np.asarray(m2), np.asarray(v2))

    key = hash((edge_index.shape[1], int(edge_index[:, ::65537].sum()),
                int(edge_index[0, :7:].sum() if edge_index.shape[1] > 7
                    else 0)))
    if key in _CACHE:
        nc, meta, idx_arrs, s_arrs = _CACHE[key]
    else:
        import time as _t
        t0 = _t.time()
        meta, idx_arrs, s_arrs = _prep(edge_index)
        print(f"[kernel] host prep: {_t.time() - t0:.1f}s", flush=True)
        t0 = _t.time()
        nc = _build(meta)
        print(f"[kernel] build+compile: {_t.time() - t0:.1f}s", flush=True)
        _CACHE.clear()
        _CACHE[key] = (nc, meta, idx_arrs, s_arrs)

    in_maps = []
    common = {
        "w1": W1p.astype(np.float16), "w2": W2p.astype(np.float16),
        "wc1": np.asarray(Wc1).astype(np.float16),
        "wc2": np.asarray(Wc2).astype(np.float16),
        "b1": b1p.reshape(-1, 1).astype(np.float32),
        "b2": b2p.reshape(-1, 1).astype(np.float32),
        "bc1": np.asarray(bc1).reshape(-1, 1).astype(np.float32),
        "bc2": np.asarray(bc2).reshape(-1, 1).astype(np.float32),
    }
    orders = meta["orders"]
    # full fp16 node table, globally degree-sorted and round-robined across
    # cores, in the chunked [chunk][core][rows] layout; identical per core.
    xf = np.zeros((TROWS, DIN), np.float16)
    prows = np.arange(SHARD)
    for c in range(NCORES):
        xf[_table_row(c, prows)] = x[orders[c]].astype(np.float16)
    dyn = (os.environ.get("GNN_DYN", "1") == "1"
           and os.environ.get("GNN_IND", "0") != "1")
    gcnts = None
    if dyn:
        GMAX = int(os.environ.get("GNN_GMAX", 512))
        R = meta["R"]
        winslots = meta["winslots"]
        rcnt = meta["rcnt"]
        gcnts = []
        for c in range(NCORES):
            cc = []
            for w in range(NWIN):
                if winslots[w] == 0:
                    continue
                for qq in range(4):
                    rq = int(R[w, qq])
                    for ck in range(0, rq, GMAX):
                        rc = min(GMAX, rq - ck)
                        cc.append(max(0, min(rc, int(rcnt[c, w, qq]) - ck)))
            while len(cc) % 8:
                cc.append(0)
            gcnts.append(np.asarray(cc, np.int32).reshape(1, -1))

    for c in range(NCORES):
        im = {"xf": xf, "idx": idx_arrs[c], "sv": s_arrs[c],
              "cinv": meta["cinv"][c].reshape(1, -1), **common}
        if dyn:
            im["gcnt"] = gcnts[c]
        in_maps.append(im)

    trace = bool(os.environ.get("GNN_TRACE"))
    res = run_bass_kernel_spmd(nc, in_maps, core_ids=list(range(NCORES)),
                               trace=trace)
    LAST_EXEC_NS = res.exec_time_ns
    if os.environ.get("GNN_BENCH"):
        LAST_EXEC_NS = _bench(nc, in_maps)

    out = np.empty((N, 2), np.float32)
    for c in range(NCORES):
        oc = res.results[c]["outT"].T[:SHARD]       # sorted order
        out[orders[c]] = oc
    return out

